# revision 2
# baseline (speedup 1.0000x reference)
"""BitNet attention (B=2, S=2048, HID=2560, NH=20, NKV=5, HD=128, GQA=4) on 8 TRN2 cores.

v2: same sharding as baseline (2-way batch x 4-way head-group tensor parallel;
core (b,g) owns q-heads [4g..4g+3, 16+g], kv slots [g, 4]), restructured for
PE stationary-weight reuse and causal narrowing:
  - j-blocks processed in phases of 2: QKV projections and o-proj loop over
    both blocks per weight tile, so consecutive matmuls share the stationary
    operand (the second LDWEIGHTS of a repeated weight is much cheaper on HW).
  - attention processes kv0's q-heads in pairs: S/AV/d matmuls for both heads
    are emitted back-to-back sharing the K/V tile stationary.
  - diagonal 128-col tiles are narrowed: S/exp/AV/d only touch the causally
    live columns; the triangle mask shrinks to a single [128,128] constant.
  - softmax denominator rows for a head pair share one PSUM bank (partitions
    0 and 32).
Host: unpack ternary weights, build RoPE tables, sum partial y / sumsq over the
4 cores of each batch, apply v/o scales and the RMSNorm per-seq scale.
"""

import math
import numpy as np
import ml_dtypes
from contextlib import ExitStack

import concourse.bacc as bacc
import concourse.tile as tile
import concourse.mybir as mybir
from concourse import bass_utils

B, S, HID = 2, 2048, 2560
NH, NKV, HD = 20, 5, 128
THETA = 500000.0
RMS_EPS = 1e-6

N_CORES = 8
KT = HID // 128          # 20 k-tiles over hidden dim
J = S // 512             # 4 seq blocks of 512
SKT = S // 128           # 16 sk tiles
NQH = 5                  # q heads per core
NKVH = 2                 # kv heads per core

F32 = mybir.dt.float32
F32R = mybir.dt.float32r
BF16 = mybir.dt.bfloat16
F16 = mybir.dt.float16

_cache = {}

DEPTH = 3  # AV flush queue depth (in tile steps)


def _build(alpha: float, repeats: int):
    nc = bacc.Bacc("TRN2", target_bir_lowering=False, debug=False, num_devices=N_CORES)

    xt_d = nc.dram_tensor("xt", [HID, S], BF16, kind="ExternalInput")
    wq_d = nc.dram_tensor("wq", [HID, NQH * HD], BF16, kind="ExternalInput")
    wk_d = nc.dram_tensor("wk", [HID, NKVH * HD], BF16, kind="ExternalInput")
    wv_d = nc.dram_tensor("wv", [HID, NKVH * HD], BF16, kind="ExternalInput")
    wo_d = nc.dram_tensor("wo", [NQH * HD, HID], BF16, kind="ExternalInput")
    cos_d = nc.dram_tensor("cos", [HD, S], F16, kind="ExternalInput")
    sin_d = nc.dram_tensor("sin", [HD, S], F16, kind="ExternalInput")
    wn_d = nc.dram_tensor("wn", [HD, NQH], F32, kind="ExternalInput")
    tri_d = nc.dram_tensor("tri", [HD, 128], BF16, kind="ExternalInput")
    onc_d = nc.dram_tensor("onc", [HD, 1], F32R, kind="ExternalInput")
    onr_d = nc.dram_tensor("onr", [1, HD], F32R, kind="ExternalInput")
    y_d = nc.dram_tensor("y", [HID, S], F32, kind="ExternalOutput")
    ssq_d = nc.dram_tensor("ssq", [1, S], F32, kind="ExternalOutput")

    with tile.TileContext(nc) as tc, ExitStack() as octx:
        ps = octx.enter_context(tc.tile_pool(name="ps", bufs=8, space="PSUM"))
        kt_p = octx.enter_context(tc.tile_pool(name="ktp", bufs=1))
        v_p = octx.enter_context(tc.tile_pool(name="vp", bufs=1))
        qb_p = octx.enter_context(tc.tile_pool(name="qbp", bufs=10))
        const_p = octx.enter_context(tc.tile_pool(name="constp", bufs=1))
        w_p = octx.enter_context(tc.tile_pool(name="wp", bufs=1))
        xt_p = octx.enter_context(tc.tile_pool(name="xtp", bufs=2))
        rp_p = octx.enter_context(tc.tile_pool(name="rpp", bufs=6))
        pr_p = octx.enter_context(tc.tile_pool(name="prp", bufs=8))
        tw_p = octx.enter_context(tc.tile_pool(name="twp", bufs=11))
        mis_p = octx.enter_context(tc.tile_pool(name="misp", bufs=4))
        y_p = octx.enter_context(tc.tile_pool(name="yp", bufs=4))

        def body(_it=None):
            # --- persistent SBUF for one iteration ---
            kt = kt_p.tile([128, NKVH * S], BF16, tag="kt", name="kt")
            vt = v_p.tile([128, SKT * NKVH * HD], BF16, tag="vt", name="vt")

            wq = w_p.tile([128, KT * NQH * HD], BF16, tag="wq", name="wq")
            wk = w_p.tile([128, KT * NKVH * HD], BF16, tag="wk", name="wk")
            wv = w_p.tile([128, KT * NKVH * HD], BF16, tag="wv", name="wv")
            wo = w_p.tile([128, NQH * HID], BF16, tag="wo", name="wo")

            def dma_w_chunk(dst, src_d, W, k0, k1):
                nc.sync.dma_start(
                    dst[:, k0 * W:k1 * W].rearrange("p (k o) -> p k o", k=k1 - k0),
                    src_d.ap()[k0 * 128:k1 * 128].rearrange("(k p) o -> p k o", p=128))

            def dma_xt_chunk(dst, j, k0, k1):
                nc.sync.dma_start(
                    dst[:, k0 * 512:k1 * 512].rearrange("p (k s) -> p k s", k=k1 - k0),
                    xt_d.ap()[k0 * 128:k1 * 128, j * 512:(j + 1) * 512]
                    .rearrange("(k p) s -> p k s", p=128))

            xts = [None] * J

            # first compute chunk's data first, then the rest interleaved
            xts[0] = xt_p.tile([128, KT * 512], BF16, tag="xt", name="xt0")
            xts[1] = xt_p.tile([128, KT * 512], BF16, tag="xt", name="xt1")
            dma_xt_chunk(xts[0], 0, 0, 5)
            dma_w_chunk(wq, wq_d, NQH * HD, 0, 5)
            dma_w_chunk(wk, wk_d, NKVH * HD, 0, 5)
            dma_w_chunk(wv, wv_d, NKVH * HD, 0, 5)
            for c in range(1, 4):
                dma_xt_chunk(xts[0], 0, 5 * c, 5 * c + 5)
                dma_w_chunk(wq, wq_d, NQH * HD, 5 * c, 5 * c + 5)
                dma_w_chunk(wk, wk_d, NKVH * HD, 5 * c, 5 * c + 5)
                dma_w_chunk(wv, wv_d, NKVH * HD, 5 * c, 5 * c + 5)
            for c in range(4):
                dma_xt_chunk(xts[1], 1, 5 * c, 5 * c + 5)

            # constants / tables
            cos_t = const_p.tile([HD, S], F16, tag="cos", name="cos")
            nc.sync.dma_start(cos_t[:], cos_d.ap())
            sin_t = const_p.tile([HD, S], F16, tag="sin", name="sin")
            nc.sync.dma_start(sin_t[:], sin_d.ap())
            onc = const_p.tile([HD, 1], F32R, tag="onc", name="onc")
            nc.sync.dma_start(onc[:], onc_d.ap())
            onr = const_p.tile([1, HD], F32R, tag="onr", name="onr")
            nc.sync.dma_start(onr[:], onr_d.ap())
            wn = const_p.tile([HD, NQH], F32, tag="wn", name="wn")
            nc.sync.dma_start(wn[:], wn_d.ap())
            tri = const_p.tile([HD, 128], BF16, tag="tri", name="tri")
            nc.sync.dma_start(tri[:], tri_d.ap())

            onc_bf = const_p.tile([HD, 1], BF16, tag="oncb", name="oncb")
            nc.any.memset(onc_bf[:], 1.0)

            # ---- tail pipeline: per-head FIFO through stages A->BC->B->C ----
            # A: drow copy (ACT) frees the packed d rows
            # BC: broadcast matmul (PE, fp32r self-loading)
            # B: reciprocal + normalize (DVE) - frees av + dbc psum slots
            # C: square (ACT) + ssq matmul (PE) + tw scale (DVE)
            pendA, pendBC, pendB, pendC = [], [], [], []

            def emit_tail_a():
                if not pendA:
                    return
                h, j, dsl, av_ps, ssq_ps, tws = pendA.pop(0)
                drow = mis_p.tile([1, 512], F32R, tag="drow", name=f"dr{j}_{h}",
                                  bufs=3)
                nc.scalar.copy(drow[:], dsl)
                pendBC.append((h, j, drow, av_ps, ssq_ps, tws))

            def emit_tail_bc():
                if not pendBC:
                    return
                h, j, drow, av_ps, ssq_ps, tws = pendBC.pop(0)
                dbc = ps.tile([128, 512], F32, tag="ps", name=f"db{j}_{h}")
                nc.tensor.matmul(dbc[:], onr[:], drow[:], start=True, stop=True)
                pendB.append((h, j, dbc, av_ps, ssq_ps, tws))

            def emit_tail_b():
                if not pendB:
                    return
                h, j, dbc, av_ps, ssq_ps, tws = pendB.pop(0)
                rec = mis_p.tile([128, 512], F32, tag="rec", name=f"rc{j}_{h}", bufs=3)
                nc.vector.reciprocal(rec[:], dbc[:])
                tn = mis_p.tile([128, 512], F32, tag="tn", name=f"tn{j}_{h}", bufs=3)
                nc.vector.tensor_mul(tn[:], av_ps[:], rec[:])
                pendC.append((h, j, tn, ssq_ps, tws))

            def emit_tail_c():
                if not pendC:
                    return
                h, j, tn, ssq_ps, tws = pendC.pop(0)
                sqt = mis_p.tile([128, 512], F32R, tag="sqt", name=f"sq{j}_{h}", bufs=3)
                nc.scalar.square(sqt[:], tn[:])
                nc.tensor.matmul(ssq_ps[:], onc[:], sqt[:],
                                 start=(h == 0), stop=(h == NQH - 1))
                tw = tw_p.tile([128, 512], BF16, tag="tw", name=f"tw{j}_{h}")
                nc.vector.tensor_scalar_mul(tw[:], tn[:], wn[:, h:h + 1])
                tws.append(tw)

            def emit_one_stage():
                # advance the oldest item one stage; at most one op per call
                if pendA:
                    emit_tail_a()
                elif pendBC:
                    emit_tail_bc()
                elif pendB:
                    emit_tail_b()
                elif pendC:
                    emit_tail_c()

            def drain_tails():
                while pendA or pendBC or pendB or pendC:
                    emit_one_stage()

            def rot_evac(psrc, nm):
                # rotate-half via two ACT cross-partition copies (PSUM->SBUF)
                qr = rp_p.tile([128, 512], F16, tag="trot", name=nm)
                nc.scalar.copy(qr[0:64, :], psrc[64:128, :])
                nc.scalar.copy(qr[64:128, :], psrc[0:64, :])
                return qr

            def rope_math2(dst, qr, sq):
                nc.vector.tensor_mul(dst, dst, cos_t[:, sq])
                nc.vector.tensor_mul(qr[:], qr[:], sin_t[:, sq])
                nc.vector.tensor_add(dst, dst, qr[:])

            # ================= attention for one block =================
            def attention_block(j, qbs, tws):
                ni = 4 * j + 4
                sq = slice(j * 512, (j + 1) * 512)
                ssq_ps = ps.tile([1, 512], F32, tag="ps", name=f"pss{j}")

                for pair in ((0, 1), (2, 3), (4,)):
                    kvl = 0 if pair[0] < 4 else 1
                    avs = [ps.tile([128, 512], F32, tag="ps", name=f"pav{j}_{h}")
                           for h in pair]
                    dt = ps.tile([128, 512], F32, tag="ps", name=f"pd{j}_{pair[0]}")
                    drows = [dt[32 * z:32 * z + 1, :] for z in range(len(pair))]
                    queue = []

                    def flush_one():
                        pi, plist = queue.pop(0)
                        st, sp = (pi == 0), (pi == ni - 1)
                        for z, (pr, cs) in enumerate(plist):
                            nc.tensor.matmul(
                                avs[z][:, cs],
                                vt[:, pi * 256 + kvl * 128: pi * 256 + kvl * 128 + 128],
                                pr[:, cs], start=st, stop=sp)
                        for z, (pr, cs) in enumerate(plist):
                            nc.tensor.matmul(
                                drows[z][:, cs], onc_bf[:], pr[:, cs],
                                start=st, stop=sp)

                    for i in range(ni):
                        o = i - 4 * j
                        cs = slice(128 * o, 512) if o > 0 else slice(0, 512)
                        plist = []
                        for z, h in enumerate(pair):
                            s_ps = ps.tile([128, 512], F32, tag="ps",
                                           name=f"pS{j}_{h}_{i}")
                            nc.tensor.matmul(
                                s_ps[:, cs],
                                kt[:, kvl * S + i * 128: kvl * S + (i + 1) * 128],
                                qbs[h][:, cs], start=True, stop=True)
                            probs = pr_p.tile([128, 512], BF16, tag="probs",
                                              name=f"pr{j}_{h}_{i}")
                            nc.scalar.activation(
                                probs[:, cs], s_ps[:, cs],
                                mybir.ActivationFunctionType.Exp, scale=alpha)
                            if o >= 0:
                                nc.vector.tensor_mul(
                                    probs[:, 128 * o:128 * o + 128],
                                    probs[:, 128 * o:128 * o + 128], tri[:])
                            plist.append((probs, cs))
                        queue.append((i, plist))
                        if len(queue) > DEPTH:
                            flush_one()
                        # tails of previous heads, up to 2 stage-ops per step
                        emit_one_stage()
                        emit_one_stage()
                    while queue:
                        flush_one()
                    for z, h in enumerate(pair):
                        pendA.append((h, j, drows[z], avs[z], ssq_ps, tws))

                return ssq_ps

            # ================= phases =================
            for p in range(2):
                js = (2 * p, 2 * p + 1)
                xpair = [xts[js[0]], xts[js[1]]]
                sqs = [slice(jj * 512, (jj + 1) * 512) for jj in js]

                # ---- Q/K projections, grouped; weight stationary shared
                # across the two blocks ----
                qbs_j = [[None] * NQH for _ in range(2)]
                for group in ((0, 1, 2), (3, 4)):
                    pss = {}
                    for m in group:
                        for t in range(2):
                            pss[(t, m)] = ps.tile([128, 512], F32, tag="ps",
                                                  name=f"pq{js[t]}_{m}")
                    for k in range(KT):
                        st, sp = (k == 0), (k == KT - 1)
                        for m in group:
                            w = wq[:, k * 640 + m * 128: k * 640 + (m + 1) * 128]
                            for t in range(2):
                                nc.tensor.matmul(
                                    pss[(t, m)][:], w,
                                    xpair[t][:, k * 512:(k + 1) * 512],
                                    start=st, stop=sp)
                        if p == 0 and group[0] == 0 and k < 8:
                            emit_one_stage()
                    if p == 0 and group[0] == 0:
                        # wo needed only at the first o-proj; start its DMA now
                        nc.sync.dma_start(
                            wo[:].rearrange("p (h o) -> p h o", h=NQH),
                            wo_d.ap().rearrange("(h p) o -> p h o", p=128))
                    for m in group:
                        for t in range(2):
                            qb = qb_p.tile([128, 512], BF16, tag="qb",
                                           name=f"qb{js[t]}_{m}")
                            nc.scalar.copy(qb[:], pss[(t, m)][:])
                            qr = rot_evac(pss[(t, m)], f"tr{js[t]}_{m}")
                            rope_math2(qb[:], qr, sqs[t])
                            qbs_j[t][m] = qb
                        emit_one_stage()

                # K projection for both blocks
                psk = {}
                for m in range(NKVH):
                    for t in range(2):
                        psk[(t, m)] = ps.tile([128, 512], F32, tag="ps",
                                              name=f"pk{js[t]}_{m}")
                for k in range(KT):
                    st, sp = (k == 0), (k == KT - 1)
                    for m in range(NKVH):
                        w = wk[:, k * 256 + m * 128: k * 256 + (m + 1) * 128]
                        for t in range(2):
                            nc.tensor.matmul(
                                psk[(t, m)][:], w,
                                xpair[t][:, k * 512:(k + 1) * 512],
                                start=st, stop=sp)
                for m in range(NKVH):
                    for t in range(2):
                        kdst = kt[:, m * S + js[t] * 512: m * S + (js[t] + 1) * 512]
                        nc.scalar.copy(kdst, psk[(t, m)][:])
                        qr = rot_evac(psk[(t, m)], f"trk{js[t]}_{m}")
                        rope_math2(kdst, qr, sqs[t])
                    emit_one_stage()

                # ---- V for both blocks (xt-stationary, per block) ----
                for t in range(2):
                    ps_v = [ps.tile([128, NKVH * HD], F32, tag="ps",
                                    name=f"pv{js[t]}_{u}")
                            for u in range(4)]
                    for k in range(KT):
                        st, sp = (k == 0), (k == KT - 1)
                        for u in range(4):
                            nc.tensor.matmul(
                                ps_v[u][:],
                                xpair[t][:, k * 512 + u * 128: k * 512 + (u + 1) * 128],
                                wv[:, k * 256:(k + 1) * 256],
                                start=st, stop=sp)
                    for u in range(4):
                        i = 4 * js[t] + u
                        nc.scalar.copy(vt[:, i * 256:(i + 1) * 256], ps_v[u][:])
                        emit_one_stage()

                # prefetch next phase's activations
                if p == 0:
                    xts[2] = xt_p.tile([128, KT * 512], BF16, tag="xt", name="xt2")
                    xts[3] = xt_p.tile([128, KT * 512], BF16, tag="xt", name="xt3")
                    for c in range(4):
                        dma_xt_chunk(xts[2], 2, 5 * c, 5 * c + 5)
                    for c in range(4):
                        dma_xt_chunk(xts[3], 3, 5 * c, 5 * c + 5)

                # ---- attention ----
                tws_j = [[], []]
                ssq_list = []
                for t in range(2):
                    ssq_list.append(attention_block(js[t], qbs_j[t], tws_j[t]))

                # ---- o-proj for both blocks, wo stationary shared ----
                first = True
                for m0 in range(0, KT, 2):
                    y_pss = {}
                    for mi in range(2):
                        for t in range(2):
                            y_pss[(t, mi)] = ps.tile([128, 512], F32, tag="ps",
                                                     name=f"py{js[t]}_{m0 + mi}")
                    for h in range(NQH):
                        if first:
                            # drain remaining tails so tws are complete
                            drain_tails()
                            for t in range(2):
                                srow = mis_p.tile([1, 512], F32, tag="srow",
                                                  name=f"sr{js[t]}", bufs=2)
                                nc.scalar.copy(srow[:], ssq_list[t][:])
                                nc.sync.dma_start(ssq_d.ap()[:, sqs[t]], srow[:])
                            first = False
                        for mi in range(2):
                            m = m0 + mi
                            w = wo[:, h * HID + m * 128: h * HID + (m + 1) * 128]
                            for t in range(2):
                                nc.tensor.matmul(
                                    y_pss[(t, mi)][:], w, tws_j[t][h][:],
                                    start=(h == 0), stop=(h == NQH - 1))
                    for mi in range(2):
                        m = m0 + mi
                        for t in range(2):
                            ysb = y_p.tile([128, 512], F32, tag="ysb",
                                           name=f"y{js[t]}_{m}")
                            if (m + t) % 2 == 0:
                                nc.scalar.copy(ysb[:], y_pss[(t, mi)][:])
                            else:
                                nc.vector.tensor_copy(ysb[:], y_pss[(t, mi)][:])
                            nc.sync.dma_start(
                                y_d.ap()[m * 128:(m + 1) * 128, sqs[t]], ysb[:])

        if repeats > 1:
            with tc.For_i(0, repeats) as _i:
                body(_i)
        else:
            body()

    nc.compile()
    return nc


def _unpack_ternary(packed: np.ndarray) -> np.ndarray:
    M, Kp = packed.shape
    nb = Kp // 32
    b = packed.reshape(M, nb, 32)
    f = np.stack([(b >> 6) & 3, (b >> 4) & 3, (b >> 2) & 3, b & 3], axis=2)
    return f.reshape(M, nb * 128).astype(np.float32) - 1.0


def _rope_tables():
    inv = 1.0 / (THETA ** (np.arange(0, HD, 2, dtype=np.float64) / HD))  # (64,)
    t = np.arange(S, dtype=np.float64)
    fr = t[None, :] * inv[:, None]          # (64, S)
    cos = np.concatenate([np.cos(fr), np.cos(fr)], axis=0)      # (128, S)
    sin = np.concatenate([-np.sin(fr), np.sin(fr)], axis=0)     # signed
    return cos.astype(np.float16), sin.astype(np.float16)


def _tri_mask():
    q = np.arange(128)[None, :]
    p = np.arange(HD)[:, None]
    return (q >= p).astype(ml_dtypes.bfloat16)


def make_in_maps(hidden_states, q_w, k_w, v_w, o_w, attn_norm_w):
    wq_f = _unpack_ternary(np.asarray(q_w))     # (2560, 2560)
    wk_f = _unpack_ternary(np.asarray(k_w))     # (640, 2560)
    wv_f = _unpack_ternary(np.asarray(v_w))     # (640, 2560)
    wo_f = _unpack_ternary(np.asarray(o_w))     # (2560, 2560) [out, in]
    cos, sin = _rope_tables()
    tri = _tri_mask()
    onc = np.ones((HD, 1), np.float32)
    onr = np.ones((1, HD), np.float32)
    wnorm = np.asarray(attn_norm_w, np.float32)
    hs = np.asarray(hidden_states)

    bf = ml_dtypes.bfloat16
    in_maps = []
    for c in range(N_CORES):
        b, g = c // 4, c % 4
        qheads = [4 * g, 4 * g + 1, 4 * g + 2, 4 * g + 3, 16 + g]
        kvheads = [g, 4]
        qrows = np.concatenate([wq_f[h * HD:(h + 1) * HD] for h in qheads], 0)
        krows = np.concatenate([wk_f[h * HD:(h + 1) * HD] for h in kvheads], 0)
        vrows = np.concatenate([wv_f[h * HD:(h + 1) * HD] for h in kvheads], 0)
        ocols = np.concatenate([wo_f[:, h * HD:(h + 1) * HD] for h in qheads], 1)
        wn = np.stack([wnorm[h * HD:(h + 1) * HD] for h in qheads], 1)  # (128, 5)
        in_maps.append({
            "xt": np.ascontiguousarray(hs[b].T).astype(bf),
            "wq": np.ascontiguousarray(qrows.T).astype(bf),
            "wk": np.ascontiguousarray(krows.T).astype(bf),
            "wv": np.ascontiguousarray(vrows.T).astype(bf),
            "wo": np.ascontiguousarray(ocols.T).astype(bf),
            "cos": cos, "sin": sin,
            "wn": np.ascontiguousarray(wn),
            "tri": tri, "onc": onc, "onr": onr,
        })
    return in_maps


def postprocess(results, v_scale, o_scale):
    out = np.empty((B, S, HID), np.float32)
    for b in range(B):
        y = np.zeros((HID, S), np.float64)
        ss = np.zeros((S,), np.float64)
        for g in range(4):
            r = results[b * 4 + g]
            y += r["y"].astype(np.float64)
            ss += r["ssq"][0].astype(np.float64)
        var = ss * (float(v_scale) ** 2) / HID
        rms = 1.0 / np.sqrt(var + RMS_EPS)
        out[b] = (y.T * (rms[:, None] * float(v_scale) * float(o_scale))).astype(np.float32)
    return out


def _get_nc(alpha: float, repeats: int = 1):
    key = (round(alpha, 12), repeats)
    if key not in _cache:
        _cache[key] = _build(alpha, repeats)
    return _cache[key]


def kernel(hidden_states, attention_mask, q_w, k_w, v_w, o_w,
           q_scale, k_scale, v_scale, o_scale, attn_norm_w):
    alpha = float(q_scale) * float(k_scale) / math.sqrt(HD)
    nc = _get_nc(alpha, 1)
    in_maps = make_in_maps(hidden_states, q_w, k_w, v_w, o_w, attn_norm_w)
    res = bass_utils.run_bass_kernel_spmd(nc, in_maps, core_ids=list(range(N_CORES)))
    return postprocess(res.results, v_scale, o_scale)


# revision 3
# speedup vs baseline: 1.0551x; 1.0551x over previous
"""BitNet attention (B=2, S=2048, HID=2560, NH=20, NKV=5, HD=128, GQA=4) on 8 TRN2 cores.

v2: same sharding as baseline (2-way batch x 4-way head-group tensor parallel;
core (b,g) owns q-heads [4g..4g+3, 16+g], kv slots [g, 4]), restructured for
PE stationary-weight reuse and causal narrowing:
  - j-blocks processed in phases of 2: QKV projections and o-proj loop over
    both blocks per weight tile, so consecutive matmuls share the stationary
    operand (the second LDWEIGHTS of a repeated weight is much cheaper on HW).
  - attention processes kv0's q-heads in pairs: S/AV/d matmuls for both heads
    are emitted back-to-back sharing the K/V tile stationary.
  - diagonal 128-col tiles are narrowed: S/exp/AV/d only touch the causally
    live columns; the triangle mask shrinks to a single [128,128] constant.
  - softmax denominator rows for a head pair share one PSUM bank (partitions
    0 and 32).
Host: unpack ternary weights, build RoPE tables, sum partial y / sumsq over the
4 cores of each batch, apply v/o scales and the RMSNorm per-seq scale.
"""

import math
import numpy as np
import ml_dtypes
from contextlib import ExitStack

import concourse.bacc as bacc
import concourse.tile as tile
import concourse.mybir as mybir
from concourse import bass_utils

B, S, HID = 2, 2048, 2560
NH, NKV, HD = 20, 5, 128
THETA = 500000.0
RMS_EPS = 1e-6

N_CORES = 8
KT = HID // 128          # 20 k-tiles over hidden dim
J = S // 512             # 4 seq blocks of 512
SKT = S // 128           # 16 sk tiles
NQH = 5                  # q heads per core
NKVH = 2                 # kv heads per core

F32 = mybir.dt.float32
F32R = mybir.dt.float32r
BF16 = mybir.dt.bfloat16
F16 = mybir.dt.float16

_cache = {}

DEPTH = 3  # AV flush queue depth (in tile steps)


def _build(alpha: float, repeats: int):
    nc = bacc.Bacc("TRN2", target_bir_lowering=False, debug=False, num_devices=N_CORES)

    xt_d = nc.dram_tensor("xt", [HID, S], BF16, kind="ExternalInput")
    wq_d = nc.dram_tensor("wq", [HID, NQH * HD], BF16, kind="ExternalInput")
    wk_d = nc.dram_tensor("wk", [HID, NKVH * HD], BF16, kind="ExternalInput")
    wv_d = nc.dram_tensor("wv", [HID, NKVH * HD], BF16, kind="ExternalInput")
    wo_d = nc.dram_tensor("wo", [NQH * HD, HID], BF16, kind="ExternalInput")
    cos_d = nc.dram_tensor("cos", [HD, S], F16, kind="ExternalInput")
    sin_d = nc.dram_tensor("sin", [HD, S], F16, kind="ExternalInput")
    wn_d = nc.dram_tensor("wn", [HD, NQH], F32, kind="ExternalInput")
    tri_d = nc.dram_tensor("tri", [HD, 128], BF16, kind="ExternalInput")
    onc_d = nc.dram_tensor("onc", [HD, 1], F32R, kind="ExternalInput")
    onr_d = nc.dram_tensor("onr", [1, HD], F32R, kind="ExternalInput")
    y_d = nc.dram_tensor("y", [HID, S], BF16, kind="ExternalOutput")
    ssq_d = nc.dram_tensor("ssq", [1, S], F32, kind="ExternalOutput")

    with tile.TileContext(nc) as tc, ExitStack() as octx:
        ps = octx.enter_context(tc.tile_pool(name="ps", bufs=8, space="PSUM"))
        kt_p = octx.enter_context(tc.tile_pool(name="ktp", bufs=1))
        v_p = octx.enter_context(tc.tile_pool(name="vp", bufs=1))
        qb_p = octx.enter_context(tc.tile_pool(name="qbp", bufs=10))
        const_p = octx.enter_context(tc.tile_pool(name="constp", bufs=1))
        w_p = octx.enter_context(tc.tile_pool(name="wp", bufs=1))
        xt_p = octx.enter_context(tc.tile_pool(name="xtp", bufs=2))
        rp_p = octx.enter_context(tc.tile_pool(name="rpp", bufs=4))
        pr_p = octx.enter_context(tc.tile_pool(name="prp", bufs=6))
        tw_p = octx.enter_context(tc.tile_pool(name="twp", bufs=21))
        mis_p = octx.enter_context(tc.tile_pool(name="misp", bufs=4))
        y_p = octx.enter_context(tc.tile_pool(name="yp", bufs=4))

        def body(_it=None):
            # --- persistent SBUF for one iteration ---
            kt = kt_p.tile([128, NKVH * S], BF16, tag="kt", name="kt")
            vt = v_p.tile([128, SKT * NKVH * HD], BF16, tag="vt", name="vt")

            wq = w_p.tile([128, KT * NQH * HD], BF16, tag="wq", name="wq")
            wk = w_p.tile([128, KT * NKVH * HD], BF16, tag="wk", name="wk")
            wv = w_p.tile([128, KT * NKVH * HD], BF16, tag="wv", name="wv")
            wo = w_p.tile([128, NQH * HID], BF16, tag="wo", name="wo")

            def dma_w_chunk(dst, src_d, W, k0, k1):
                nc.sync.dma_start(
                    dst[:, k0 * W:k1 * W].rearrange("p (k o) -> p k o", k=k1 - k0),
                    src_d.ap()[k0 * 128:k1 * 128].rearrange("(k p) o -> p k o", p=128))

            def dma_xt_chunk(dst, j, k0, k1):
                nc.sync.dma_start(
                    dst[:, k0 * 512:k1 * 512].rearrange("p (k s) -> p k s", k=k1 - k0),
                    xt_d.ap()[k0 * 128:k1 * 128, j * 512:(j + 1) * 512]
                    .rearrange("(k p) s -> p k s", p=128))

            xts = [None] * J

            # first compute chunk's data first, then the rest interleaved
            xts[0] = xt_p.tile([128, KT * 512], BF16, tag="xt", name="xt0")
            xts[1] = xt_p.tile([128, KT * 512], BF16, tag="xt", name="xt1")
            dma_xt_chunk(xts[0], 0, 0, 5)
            dma_w_chunk(wq, wq_d, NQH * HD, 0, 5)
            dma_w_chunk(wk, wk_d, NKVH * HD, 0, 5)
            dma_w_chunk(wv, wv_d, NKVH * HD, 0, 5)
            for c in range(1, 4):
                dma_xt_chunk(xts[0], 0, 5 * c, 5 * c + 5)
                dma_w_chunk(wq, wq_d, NQH * HD, 5 * c, 5 * c + 5)
                dma_w_chunk(wk, wk_d, NKVH * HD, 5 * c, 5 * c + 5)
                dma_w_chunk(wv, wv_d, NKVH * HD, 5 * c, 5 * c + 5)
            for c in range(4):
                dma_xt_chunk(xts[1], 1, 5 * c, 5 * c + 5)

            # constants / tables
            cos_t = const_p.tile([HD, S], F16, tag="cos", name="cos")
            nc.sync.dma_start(cos_t[:], cos_d.ap())
            sin_t = const_p.tile([HD, S], F16, tag="sin", name="sin")
            nc.sync.dma_start(sin_t[:], sin_d.ap())
            onc = const_p.tile([HD, 1], F32R, tag="onc", name="onc")
            nc.sync.dma_start(onc[:], onc_d.ap())
            onr = const_p.tile([1, HD], F32R, tag="onr", name="onr")
            nc.sync.dma_start(onr[:], onr_d.ap())
            wn = const_p.tile([HD, NQH], F32, tag="wn", name="wn")
            nc.sync.dma_start(wn[:], wn_d.ap())
            tri = const_p.tile([HD, 128], BF16, tag="tri", name="tri")
            nc.sync.dma_start(tri[:], tri_d.ap())

            onc_bf = const_p.tile([HD, 1], BF16, tag="oncb", name="oncb")
            nc.any.memset(onc_bf[:], 1.0)

            # ---- tail pipeline: per-head FIFO through stages A->BC->B->C ----
            # A: drow copy (ACT) frees the packed d rows
            # BC: broadcast matmul (PE, fp32r self-loading)
            # B: reciprocal + normalize (DVE) - frees av + dbc psum slots
            # C: square (ACT) + ssq matmul (PE) + tw scale (DVE)
            pendA, pendBC, pendB, pendC = [], [], [], []

            def emit_tail_a():
                if not pendA:
                    return
                h, j, dsl, av_ps, ssq_ps, tws = pendA.pop(0)
                drow = mis_p.tile([1, 512], F32R, tag="drow", name=f"dr{j}_{h}",
                                  bufs=2)
                nc.scalar.copy(drow[:], dsl)
                pendBC.append((h, j, drow, av_ps, ssq_ps, tws))

            def emit_tail_bc():
                if not pendBC:
                    return
                h, j, drow, av_ps, ssq_ps, tws = pendBC.pop(0)
                dbc = ps.tile([128, 512], F32, tag="ps", name=f"db{j}_{h}")
                nc.tensor.matmul(dbc[:], onr[:], drow[:], start=True, stop=True)
                pendB.append((h, j, dbc, av_ps, ssq_ps, tws))

            def emit_tail_b():
                if not pendB:
                    return
                h, j, dbc, av_ps, ssq_ps, tws = pendB.pop(0)
                rec = mis_p.tile([128, 512], F32, tag="rec", name=f"rc{j}_{h}", bufs=2)
                nc.vector.reciprocal(rec[:], dbc[:])
                tn = mis_p.tile([128, 512], F32, tag="tn", name=f"tn{j}_{h}", bufs=2)
                nc.vector.tensor_mul(tn[:], av_ps[:], rec[:])
                pendC.append((h, j, tn, ssq_ps, tws))

            def emit_tail_c():
                if not pendC:
                    return
                h, j, tn, ssq_ps, tws = pendC.pop(0)
                sqt = mis_p.tile([128, 512], F32R, tag="sqt", name=f"sq{j}_{h}", bufs=2)
                nc.scalar.square(sqt[:], tn[:])
                nc.tensor.matmul(ssq_ps[:], onc[:], sqt[:],
                                 start=(h == 0), stop=(h == NQH - 1))
                tw = tw_p.tile([128, 512], BF16, tag="tw", name=f"tw{j}_{h}")
                nc.vector.tensor_scalar_mul(tw[:], tn[:], wn[:, h:h + 1])
                tws.append(tw)

            def emit_one_stage():
                # advance the oldest item one stage; at most one op per call
                if pendA:
                    emit_tail_a()
                elif pendBC:
                    emit_tail_bc()
                elif pendB:
                    emit_tail_b()
                elif pendC:
                    emit_tail_c()

            def drain_tails():
                while pendA or pendBC or pendB or pendC:
                    emit_one_stage()

            def rot_evac(psrc, nm):
                # rotate-half via two ACT cross-partition copies (PSUM->SBUF)
                qr = rp_p.tile([128, 512], F16, tag="trot", name=nm)
                nc.scalar.copy(qr[0:64, :], psrc[64:128, :])
                nc.scalar.copy(qr[64:128, :], psrc[0:64, :])
                return qr

            def rope_math2(dst, qr, sq):
                nc.vector.tensor_mul(dst, dst, cos_t[:, sq])
                nc.vector.tensor_mul(qr[:], qr[:], sin_t[:, sq])
                nc.vector.tensor_add(dst, dst, qr[:])

            # ================= attention for one block =================
            def attention_block(j, qbs, tws, emit_extra=None):
                ni = 4 * j + 4
                sq = slice(j * 512, (j + 1) * 512)
                ssq_ps = ps.tile([1, 512], F32, tag="ps", name=f"pss{j}")

                for pair in ((0, 1), (2, 3), (4,)):
                    kvl = 0 if pair[0] < 4 else 1
                    avs = [ps.tile([128, 512], F32, tag="ps", name=f"pav{j}_{h}")
                           for h in pair]
                    dt = ps.tile([128, 512], F32, tag="ps", name=f"pd{j}_{pair[0]}")
                    drows = [dt[32 * z:32 * z + 1, :] for z in range(len(pair))]
                    queue = []

                    def flush_one():
                        pi, plist = queue.pop(0)
                        st, sp = (pi == 0), (pi == ni - 1)
                        for z, (pr, cs) in enumerate(plist):
                            nc.tensor.matmul(
                                avs[z][:, cs],
                                vt[:, pi * 256 + kvl * 128: pi * 256 + kvl * 128 + 128],
                                pr[:, cs], start=st, stop=sp)
                        for z, (pr, cs) in enumerate(plist):
                            nc.tensor.matmul(
                                drows[z][:, cs], onc_bf[:], pr[:, cs],
                                start=st, stop=sp)

                    for i in range(ni):
                        o = i - 4 * j
                        cs = slice(128 * o, 512) if o > 0 else slice(0, 512)
                        plist = []
                        for z, h in enumerate(pair):
                            s_ps = ps.tile([128, 512], F32, tag="ps",
                                           name=f"pS{j}_{h}_{i}")
                            nc.tensor.matmul(
                                s_ps[:, cs],
                                kt[:, kvl * S + i * 128: kvl * S + (i + 1) * 128],
                                qbs[h][:, cs], start=True, stop=True)
                            probs = pr_p.tile([128, 512], BF16, tag="probs",
                                              name=f"pr{j}_{h}_{i}")
                            nc.scalar.activation(
                                probs[:, cs], s_ps[:, cs],
                                mybir.ActivationFunctionType.Exp, scale=alpha)
                            if o >= 0:
                                nc.vector.tensor_mul(
                                    probs[:, 128 * o:128 * o + 128],
                                    probs[:, 128 * o:128 * o + 128], tri[:])
                            plist.append((probs, cs))
                        queue.append((i, plist))
                        if len(queue) > DEPTH:
                            flush_one()
                        # tails of previous heads, up to 2 stage-ops per step
                        emit_one_stage()
                        emit_one_stage()
                        if emit_extra is not None:
                            emit_extra()
                    while queue:
                        flush_one()
                    for z, h in enumerate(pair):
                        pendA.append((h, j, drows[z], avs[z], ssq_ps, tws))

                return ssq_ps

            # ================= phases =================
            pend_oproj = []
            pend_ssq_flush = None
            for p in range(2):
                js = (2 * p, 2 * p + 1)
                xpair = [xts[js[0]], xts[js[1]]]
                sqs = [slice(jj * 512, (jj + 1) * 512) for jj in js]

                # ---- Q/K projections, grouped; weight stationary shared
                # across the two blocks ----
                qbs_j = [[None] * NQH for _ in range(2)]
                for group in ((0, 1, 2), (3, 4)):
                    pss = {}
                    for m in group:
                        for t in range(2):
                            pss[(t, m)] = ps.tile([128, 512], F32, tag="ps",
                                                  name=f"pq{js[t]}_{m}")
                    for k in range(KT):
                        st, sp = (k == 0), (k == KT - 1)
                        for m in group:
                            w = wq[:, k * 640 + m * 128: k * 640 + (m + 1) * 128]
                            for t in range(2):
                                nc.tensor.matmul(
                                    pss[(t, m)][:], w,
                                    xpair[t][:, k * 512:(k + 1) * 512],
                                    start=st, stop=sp)
                        if p == 0 and group[0] == 0 and k < 8:
                            emit_one_stage()
                    if p == 0 and group[0] == 0:
                        # wo needed only at the first o-proj; start its DMA now
                        nc.sync.dma_start(
                            wo[:].rearrange("p (h o) -> p h o", h=NQH),
                            wo_d.ap().rearrange("(h p) o -> p h o", p=128))
                    for m in group:
                        for t in range(2):
                            qb = qb_p.tile([128, 512], BF16, tag="qb",
                                           name=f"qb{js[t]}_{m}")
                            nc.vector.tensor_copy(qb[:], pss[(t, m)][:])
                            qr = rot_evac(pss[(t, m)], f"tr{js[t]}_{m}")
                            rope_math2(qb[:], qr, sqs[t])
                            qbs_j[t][m] = qb
                        emit_one_stage()

                # K projection for both blocks
                psk = {}
                for m in range(NKVH):
                    for t in range(2):
                        psk[(t, m)] = ps.tile([128, 512], F32, tag="ps",
                                              name=f"pk{js[t]}_{m}")
                for k in range(KT):
                    st, sp = (k == 0), (k == KT - 1)
                    for m in range(NKVH):
                        w = wk[:, k * 256 + m * 128: k * 256 + (m + 1) * 128]
                        for t in range(2):
                            nc.tensor.matmul(
                                psk[(t, m)][:], w,
                                xpair[t][:, k * 512:(k + 1) * 512],
                                start=st, stop=sp)
                for m in range(NKVH):
                    for t in range(2):
                        kdst = kt[:, m * S + js[t] * 512: m * S + (js[t] + 1) * 512]
                        nc.scalar.copy(kdst, psk[(t, m)][:])
                        qr = rot_evac(psk[(t, m)], f"trk{js[t]}_{m}")
                        rope_math2(kdst, qr, sqs[t])
                    emit_one_stage()

                # ---- V for both blocks (xt-stationary, per block) ----
                for t in range(2):
                    ps_v = [ps.tile([128, NKVH * HD], F32, tag="ps",
                                    name=f"pv{js[t]}_{u}")
                            for u in range(4)]
                    for k in range(KT):
                        st, sp = (k == 0), (k == KT - 1)
                        for u in range(4):
                            nc.tensor.matmul(
                                ps_v[u][:],
                                xpair[t][:, k * 512 + u * 128: k * 512 + (u + 1) * 128],
                                wv[:, k * 256:(k + 1) * 256],
                                start=st, stop=sp)
                    for u in range(4):
                        i = 4 * js[t] + u
                        nc.scalar.copy(vt[:, i * 256:(i + 1) * 256], ps_v[u][:])
                        emit_one_stage()

                # prefetch next phase's activations
                if p == 0:
                    xts[2] = xt_p.tile([128, KT * 512], BF16, tag="xt", name="xt2")
                    xts[3] = xt_p.tile([128, KT * 512], BF16, tag="xt", name="xt3")
                    for c in range(4):
                        dma_xt_chunk(xts[2], 2, 5 * c, 5 * c + 5)
                    for c in range(4):
                        dma_xt_chunk(xts[3], 3, 5 * c, 5 * c + 5)

                # flush the previous phase's ssq (frees its PSUM slots) and
                # interleave its o-proj chunks into this phase's attention
                if pend_oproj:
                    drain_tails()
                    pend_ssq_flush()

                def emit_oproj():
                    if pend_oproj:
                        pend_oproj.pop(0)()

                # ---- attention ----
                tws_j = [[], []]
                ssq_list = []
                for t in range(2):
                    ssq_list.append(attention_block(js[t], qbs_j[t], tws_j[t],
                                                    emit_oproj))
                while pend_oproj:
                    pend_oproj.pop(0)()

                # ---- o-proj chunk closures for this phase (wo shared) ----
                def mk_ssq_flush(ssqs, sqs_):
                    def f():
                        for t in range(2):
                            srow = mis_p.tile([1, 512], F32, tag="srow",
                                              name=f"sr{sqs_[t].start}", bufs=2)
                            nc.scalar.copy(srow[:], ssqs[t][:])
                            nc.sync.dma_start(ssq_d.ap()[:, sqs_[t]], srow[:])
                    return f

                pend_ssq_flush = mk_ssq_flush(ssq_list, sqs)

                def mk_chunk(m, tws_, sqs_):
                    def f():
                        y_pss = [ps.tile([128, 512], F32, tag="ps",
                                         name=f"py{sqs_[t].start}_{m}")
                                 for t in range(2)]
                        for h in range(NQH):
                            w = wo[:, h * HID + m * 128: h * HID + (m + 1) * 128]
                            for t in range(2):
                                nc.tensor.matmul(
                                    y_pss[t][:], w, tws_[t][h][:],
                                    start=(h == 0), stop=(h == NQH - 1))
                        for t in range(2):
                            ysb = y_p.tile([128, 512], BF16, tag="ysb",
                                           name=f"y{sqs_[t].start}_{m}")
                            if (m + t) % 2 == 0:
                                nc.scalar.copy(ysb[:], y_pss[t][:])
                            else:
                                nc.vector.tensor_copy(ysb[:], y_pss[t][:])
                            nc.sync.dma_start(
                                y_d.ap()[m * 128:(m + 1) * 128, sqs_[t]], ysb[:])
                    return f

                pend_oproj = [mk_chunk(m, tws_j, sqs) for m in range(KT)]
                if p == 1:
                    # last phase: nothing to hide under; run now
                    drain_tails()
                    pend_ssq_flush()
                    while pend_oproj:
                        pend_oproj.pop(0)()

        if repeats > 1:
            with tc.For_i(0, repeats) as _i:
                body(_i)
        else:
            body()

    nc.compile()
    return nc


def _unpack_ternary(packed: np.ndarray) -> np.ndarray:
    M, Kp = packed.shape
    nb = Kp // 32
    b = packed.reshape(M, nb, 32)
    f = np.stack([(b >> 6) & 3, (b >> 4) & 3, (b >> 2) & 3, b & 3], axis=2)
    return f.reshape(M, nb * 128).astype(np.float32) - 1.0


def _rope_tables():
    inv = 1.0 / (THETA ** (np.arange(0, HD, 2, dtype=np.float64) / HD))  # (64,)
    t = np.arange(S, dtype=np.float64)
    fr = t[None, :] * inv[:, None]          # (64, S)
    cos = np.concatenate([np.cos(fr), np.cos(fr)], axis=0)      # (128, S)
    sin = np.concatenate([-np.sin(fr), np.sin(fr)], axis=0)     # signed
    return cos.astype(np.float16), sin.astype(np.float16)


def _tri_mask():
    q = np.arange(128)[None, :]
    p = np.arange(HD)[:, None]
    return (q >= p).astype(ml_dtypes.bfloat16)


def make_in_maps(hidden_states, q_w, k_w, v_w, o_w, attn_norm_w):
    wq_f = _unpack_ternary(np.asarray(q_w))     # (2560, 2560)
    wk_f = _unpack_ternary(np.asarray(k_w))     # (640, 2560)
    wv_f = _unpack_ternary(np.asarray(v_w))     # (640, 2560)
    wo_f = _unpack_ternary(np.asarray(o_w))     # (2560, 2560) [out, in]
    cos, sin = _rope_tables()
    tri = _tri_mask()
    onc = np.ones((HD, 1), np.float32)
    onr = np.ones((1, HD), np.float32)
    wnorm = np.asarray(attn_norm_w, np.float32)
    hs = np.asarray(hidden_states)

    bf = ml_dtypes.bfloat16
    in_maps = []
    for c in range(N_CORES):
        b, g = c // 4, c % 4
        qheads = [4 * g, 4 * g + 1, 4 * g + 2, 4 * g + 3, 16 + g]
        kvheads = [g, 4]
        qrows = np.concatenate([wq_f[h * HD:(h + 1) * HD] for h in qheads], 0)
        krows = np.concatenate([wk_f[h * HD:(h + 1) * HD] for h in kvheads], 0)
        vrows = np.concatenate([wv_f[h * HD:(h + 1) * HD] for h in kvheads], 0)
        ocols = np.concatenate([wo_f[:, h * HD:(h + 1) * HD] for h in qheads], 1)
        wn = np.stack([wnorm[h * HD:(h + 1) * HD] for h in qheads], 1)  # (128, 5)
        in_maps.append({
            "xt": np.ascontiguousarray(hs[b].T).astype(bf),
            "wq": np.ascontiguousarray(qrows.T).astype(bf),
            "wk": np.ascontiguousarray(krows.T).astype(bf),
            "wv": np.ascontiguousarray(vrows.T).astype(bf),
            "wo": np.ascontiguousarray(ocols.T).astype(bf),
            "cos": cos, "sin": sin,
            "wn": np.ascontiguousarray(wn),
            "tri": tri, "onc": onc, "onr": onr,
        })
    return in_maps


def postprocess(results, v_scale, o_scale):
    out = np.empty((B, S, HID), np.float32)
    for b in range(B):
        y = np.zeros((HID, S), np.float64)
        ss = np.zeros((S,), np.float64)
        for g in range(4):
            r = results[b * 4 + g]
            y += r["y"].astype(np.float64)
            ss += r["ssq"][0].astype(np.float64)
        var = ss * (float(v_scale) ** 2) / HID
        rms = 1.0 / np.sqrt(var + RMS_EPS)
        out[b] = (y.T * (rms[:, None] * float(v_scale) * float(o_scale))).astype(np.float32)
    return out


def _get_nc(alpha: float, repeats: int = 1):
    key = (round(alpha, 12), repeats)
    if key not in _cache:
        _cache[key] = _build(alpha, repeats)
    return _cache[key]


def kernel(hidden_states, attention_mask, q_w, k_w, v_w, o_w,
           q_scale, k_scale, v_scale, o_scale, attn_norm_w):
    alpha = float(q_scale) * float(k_scale) / math.sqrt(HD)
    nc = _get_nc(alpha, 1)
    in_maps = make_in_maps(hidden_states, q_w, k_w, v_w, o_w, attn_norm_w)
    res = bass_utils.run_bass_kernel_spmd(nc, in_maps, core_ids=list(range(N_CORES)))
    return postprocess(res.results, v_scale, o_scale)


# revision 4
# speedup vs baseline: 1.0568x; 1.0016x over previous
"""BitNet attention (B=2, S=2048, HID=2560, NH=20, NKV=5, HD=128, GQA=4) on 8 TRN2 cores.

v2: same sharding as baseline (2-way batch x 4-way head-group tensor parallel;
core (b,g) owns q-heads [4g..4g+3, 16+g], kv slots [g, 4]), restructured for
PE stationary-weight reuse and causal narrowing:
  - j-blocks processed in phases of 2: QKV projections and o-proj loop over
    both blocks per weight tile, so consecutive matmuls share the stationary
    operand (the second LDWEIGHTS of a repeated weight is much cheaper on HW).
  - attention processes kv0's q-heads in pairs: S/AV/d matmuls for both heads
    are emitted back-to-back sharing the K/V tile stationary.
  - diagonal 128-col tiles are narrowed: S/exp/AV/d only touch the causally
    live columns; the triangle mask shrinks to a single [128,128] constant.
  - softmax denominator rows for a head pair share one PSUM bank (partitions
    0 and 32).
Host: unpack ternary weights, build RoPE tables, sum partial y / sumsq over the
4 cores of each batch, apply v/o scales and the RMSNorm per-seq scale.
"""

import math
import numpy as np
import ml_dtypes
from contextlib import ExitStack

import concourse.bacc as bacc
import concourse.tile as tile
import concourse.mybir as mybir
from concourse import bass_utils

B, S, HID = 2, 2048, 2560
NH, NKV, HD = 20, 5, 128
THETA = 500000.0
RMS_EPS = 1e-6

N_CORES = 8
KT = HID // 128          # 20 k-tiles over hidden dim
J = S // 512             # 4 seq blocks of 512
SKT = S // 128           # 16 sk tiles
NQH = 5                  # q heads per core
NKVH = 2                 # kv heads per core

F32 = mybir.dt.float32
F32R = mybir.dt.float32r
BF16 = mybir.dt.bfloat16
F16 = mybir.dt.float16

_cache = {}

DEPTH = 3  # AV flush queue depth (in tile steps)


def _build(alpha: float, repeats: int):
    nc = bacc.Bacc("TRN2", target_bir_lowering=False, debug=False, num_devices=N_CORES)

    xt_d = nc.dram_tensor("xt", [HID, S], BF16, kind="ExternalInput")
    wq_d = nc.dram_tensor("wq", [HID, NQH * HD], BF16, kind="ExternalInput")
    wk_d = nc.dram_tensor("wk", [HID, NKVH * HD], BF16, kind="ExternalInput")
    wv_d = nc.dram_tensor("wv", [HID, NKVH * HD], BF16, kind="ExternalInput")
    wo_d = nc.dram_tensor("wo", [NQH * HD, HID], BF16, kind="ExternalInput")
    cos_d = nc.dram_tensor("cos", [HD, S], F16, kind="ExternalInput")
    sin_d = nc.dram_tensor("sin", [HD, S], F16, kind="ExternalInput")
    wn_d = nc.dram_tensor("wn", [HD, NQH], F32, kind="ExternalInput")
    tri_d = nc.dram_tensor("tri", [HD, 128], BF16, kind="ExternalInput")
    onc_d = nc.dram_tensor("onc", [HD, 1], F32R, kind="ExternalInput")
    onr_d = nc.dram_tensor("onr", [1, HD], F32R, kind="ExternalInput")
    y_d = nc.dram_tensor("y", [HID, S], BF16, kind="ExternalOutput")
    ssq_d = nc.dram_tensor("ssq", [1, S], F32, kind="ExternalOutput")

    with tile.TileContext(nc) as tc, ExitStack() as octx:
        ps = octx.enter_context(tc.tile_pool(name="ps", bufs=8, space="PSUM"))
        kt_p = octx.enter_context(tc.tile_pool(name="ktp", bufs=1))
        v_p = octx.enter_context(tc.tile_pool(name="vp", bufs=1))
        qb_p = octx.enter_context(tc.tile_pool(name="qbp", bufs=10))
        const_p = octx.enter_context(tc.tile_pool(name="constp", bufs=1))
        w_p = octx.enter_context(tc.tile_pool(name="wp", bufs=1))
        xt_p = octx.enter_context(tc.tile_pool(name="xtp", bufs=2))
        rp_p = octx.enter_context(tc.tile_pool(name="rpp", bufs=4))
        pr_p = octx.enter_context(tc.tile_pool(name="prp", bufs=6))
        tw_p = octx.enter_context(tc.tile_pool(name="twp", bufs=21))
        mis_p = octx.enter_context(tc.tile_pool(name="misp", bufs=4))
        y_p = octx.enter_context(tc.tile_pool(name="yp", bufs=4))

        def dma_w_chunk(dst, src_d, W, k0, k1):
            nc.sync.dma_start(
                dst[:, k0 * W:k1 * W].rearrange("p (k o) -> p k o", k=k1 - k0),
                src_d.ap()[k0 * 128:k1 * 128].rearrange("(k p) o -> p k o", p=128))

        # ---- prologue: weights + constants loaded once before the loop;
        # inside the loop the weight tiles are re-filled late (after their
        # last reader) so iteration N+1's loads overlap iteration N's tail.
        wq = w_p.tile([128, KT * NQH * HD], BF16, tag="wq", name="wq")
        wk = w_p.tile([128, KT * NKVH * HD], BF16, tag="wk", name="wk")
        wv = w_p.tile([128, KT * NKVH * HD], BF16, tag="wv", name="wv")
        wo = w_p.tile([128, NQH * HID], BF16, tag="wo", name="wo")
        for c in range(4):
            dma_w_chunk(wq, wq_d, NQH * HD, 5 * c, 5 * c + 5)
            dma_w_chunk(wk, wk_d, NKVH * HD, 5 * c, 5 * c + 5)
            dma_w_chunk(wv, wv_d, NKVH * HD, 5 * c, 5 * c + 5)
        nc.sync.dma_start(
            wo[:].rearrange("p (h o) -> p h o", h=NQH),
            wo_d.ap().rearrange("(h p) o -> p h o", p=128))
        cos_t = const_p.tile([HD, S], F16, tag="cos", name="cos")
        nc.sync.dma_start(cos_t[:], cos_d.ap())
        sin_t = const_p.tile([HD, S], F16, tag="sin", name="sin")
        nc.sync.dma_start(sin_t[:], sin_d.ap())
        onc = const_p.tile([HD, 1], F32R, tag="onc", name="onc")
        nc.sync.dma_start(onc[:], onc_d.ap())
        onr = const_p.tile([1, HD], F32R, tag="onr", name="onr")
        nc.sync.dma_start(onr[:], onr_d.ap())
        wn = const_p.tile([HD, NQH], F32, tag="wn", name="wn")
        nc.sync.dma_start(wn[:], wn_d.ap())
        tri = const_p.tile([HD, 128], BF16, tag="tri", name="tri")
        nc.sync.dma_start(tri[:], tri_d.ap())
        onc_bf = const_p.tile([HD, 1], BF16, tag="oncb", name="oncb")
        nc.any.memset(onc_bf[:], 1.0)

        def body(_it=None):
            # --- per-iteration SBUF ---
            kt = kt_p.tile([128, NKVH * S], BF16, tag="kt", name="kt")
            vt = v_p.tile([128, SKT * NKVH * HD], BF16, tag="vt", name="vt")

            def dma_xt_chunk(dst, j, k0, k1):
                nc.sync.dma_start(
                    dst[:, k0 * 512:k1 * 512].rearrange("p (k s) -> p k s", k=k1 - k0),
                    xt_d.ap()[k0 * 128:k1 * 128, j * 512:(j + 1) * 512]
                    .rearrange("(k p) s -> p k s", p=128))

            xts = [None] * J
            xts[0] = xt_p.tile([128, KT * 512], BF16, tag="xt", name="xt0")
            xts[1] = xt_p.tile([128, KT * 512], BF16, tag="xt", name="xt1")
            for c in range(4):
                dma_xt_chunk(xts[0], 0, 5 * c, 5 * c + 5)
            for c in range(4):
                dma_xt_chunk(xts[1], 1, 5 * c, 5 * c + 5)

            # ---- tail pipeline: per-head FIFO through stages A->BC->B->C ----
            # A: drow copy (ACT) frees the packed d rows
            # BC: broadcast matmul (PE, fp32r self-loading)
            # B: reciprocal + normalize (DVE) - frees av + dbc psum slots
            # C: square (ACT) + ssq matmul (PE) + tw scale (DVE)
            pendA, pendBC, pendB, pendC = [], [], [], []

            def emit_tail_a():
                if not pendA:
                    return
                h, j, dsl, av_ps, ssq_ps, tws = pendA.pop(0)
                drow = mis_p.tile([1, 512], F32R, tag="drow", name=f"dr{j}_{h}",
                                  bufs=2)
                nc.scalar.copy(drow[:], dsl)
                pendBC.append((h, j, drow, av_ps, ssq_ps, tws))

            def emit_tail_bc():
                if not pendBC:
                    return
                h, j, drow, av_ps, ssq_ps, tws = pendBC.pop(0)
                dbc = ps.tile([128, 512], F32, tag="ps", name=f"db{j}_{h}")
                nc.tensor.matmul(dbc[:], onr[:], drow[:], start=True, stop=True)
                pendB.append((h, j, dbc, av_ps, ssq_ps, tws))

            def emit_tail_b():
                if not pendB:
                    return
                h, j, dbc, av_ps, ssq_ps, tws = pendB.pop(0)
                rec = mis_p.tile([128, 512], F32, tag="rec", name=f"rc{j}_{h}", bufs=2)
                nc.vector.reciprocal(rec[:], dbc[:])
                tn = mis_p.tile([128, 512], F32, tag="tn", name=f"tn{j}_{h}", bufs=2)
                nc.vector.tensor_mul(tn[:], av_ps[:], rec[:])
                pendC.append((h, j, tn, ssq_ps, tws))

            def emit_tail_c():
                if not pendC:
                    return
                h, j, tn, ssq_ps, tws = pendC.pop(0)
                sqt = mis_p.tile([128, 512], F32R, tag="sqt", name=f"sq{j}_{h}", bufs=2)
                nc.scalar.square(sqt[:], tn[:])
                nc.tensor.matmul(ssq_ps[:], onc[:], sqt[:],
                                 start=(h == 0), stop=(h == NQH - 1))
                tw = tw_p.tile([128, 512], BF16, tag="tw", name=f"tw{j}_{h}")
                nc.vector.tensor_scalar_mul(tw[:], tn[:], wn[:, h:h + 1])
                tws.append(tw)

            def emit_one_stage():
                # advance the oldest item one stage; at most one op per call
                if pendA:
                    emit_tail_a()
                elif pendBC:
                    emit_tail_bc()
                elif pendB:
                    emit_tail_b()
                elif pendC:
                    emit_tail_c()

            def drain_tails():
                while pendA or pendBC or pendB or pendC:
                    emit_one_stage()

            def rot_evac(psrc, nm):
                # rotate-half via two ACT cross-partition copies (PSUM->SBUF)
                qr = rp_p.tile([128, 512], F16, tag="trot", name=nm)
                nc.scalar.copy(qr[0:64, :], psrc[64:128, :])
                nc.scalar.copy(qr[64:128, :], psrc[0:64, :])
                return qr

            def rope_math2(dst, qr, sq):
                nc.vector.tensor_mul(dst, dst, cos_t[:, sq])
                nc.vector.tensor_mul(qr[:], qr[:], sin_t[:, sq])
                nc.vector.tensor_add(dst, dst, qr[:])

            # ================= attention for one block =================
            def attention_block(j, qbs, tws, emit_extra=None):
                ni = 4 * j + 4
                sq = slice(j * 512, (j + 1) * 512)
                ssq_ps = ps.tile([1, 512], F32, tag="ps", name=f"pss{j}")

                for pair in ((0, 1), (2, 3), (4,)):
                    kvl = 0 if pair[0] < 4 else 1
                    avs = [ps.tile([128, 512], F32, tag="ps", name=f"pav{j}_{h}")
                           for h in pair]
                    dt = ps.tile([128, 512], F32, tag="ps", name=f"pd{j}_{pair[0]}")
                    drows = [dt[32 * z:32 * z + 1, :] for z in range(len(pair))]
                    queue = []

                    def flush_one():
                        pi, plist = queue.pop(0)
                        st, sp = (pi == 0), (pi == ni - 1)
                        for z, (pr, cs) in enumerate(plist):
                            nc.tensor.matmul(
                                avs[z][:, cs],
                                vt[:, pi * 256 + kvl * 128: pi * 256 + kvl * 128 + 128],
                                pr[:, cs], start=st, stop=sp)
                        for z, (pr, cs) in enumerate(plist):
                            nc.tensor.matmul(
                                drows[z][:, cs], onc_bf[:], pr[:, cs],
                                start=st, stop=sp)

                    for i in range(ni):
                        o = i - 4 * j
                        cs = slice(128 * o, 512) if o > 0 else slice(0, 512)
                        plist = []
                        for z, h in enumerate(pair):
                            s_ps = ps.tile([128, 512], F32, tag="ps",
                                           name=f"pS{j}_{h}_{i}")
                            nc.tensor.matmul(
                                s_ps[:, cs],
                                kt[:, kvl * S + i * 128: kvl * S + (i + 1) * 128],
                                qbs[h][:, cs], start=True, stop=True)
                            probs = pr_p.tile([128, 512], BF16, tag="probs",
                                              name=f"pr{j}_{h}_{i}")
                            nc.scalar.activation(
                                probs[:, cs], s_ps[:, cs],
                                mybir.ActivationFunctionType.Exp, scale=alpha)
                            if o >= 0:
                                nc.vector.tensor_mul(
                                    probs[:, 128 * o:128 * o + 128],
                                    probs[:, 128 * o:128 * o + 128], tri[:])
                            plist.append((probs, cs))
                        queue.append((i, plist))
                        if len(queue) > DEPTH:
                            flush_one()
                        # tails of previous heads, up to 2 stage-ops per step
                        emit_one_stage()
                        emit_one_stage()
                        if emit_extra is not None:
                            emit_extra()
                    while queue:
                        flush_one()
                    for z, h in enumerate(pair):
                        pendA.append((h, j, drows[z], avs[z], ssq_ps, tws))

                return ssq_ps

            # ================= phases =================
            pend_oproj = []
            pend_ssq_flush = None
            for p in range(2):
                js = (2 * p, 2 * p + 1)
                xpair = [xts[js[0]], xts[js[1]]]
                sqs = [slice(jj * 512, (jj + 1) * 512) for jj in js]

                # ---- Q/K projections, grouped; weight stationary shared
                # across the two blocks ----
                qbs_j = [[None] * NQH for _ in range(2)]
                for group in ((0, 1, 2), (3, 4)):
                    pss = {}
                    for m in group:
                        for t in range(2):
                            pss[(t, m)] = ps.tile([128, 512], F32, tag="ps",
                                                  name=f"pq{js[t]}_{m}")
                    for k in range(KT):
                        st, sp = (k == 0), (k == KT - 1)
                        for m in group:
                            w = wq[:, k * 640 + m * 128: k * 640 + (m + 1) * 128]
                            for t in range(2):
                                nc.tensor.matmul(
                                    pss[(t, m)][:], w,
                                    xpair[t][:, k * 512:(k + 1) * 512],
                                    start=st, stop=sp)
                        if p == 0 and group[0] == 0 and k < 8:
                            emit_one_stage()
                    for m in group:
                        for t in range(2):
                            qb = qb_p.tile([128, 512], BF16, tag="qb",
                                           name=f"qb{js[t]}_{m}")
                            nc.vector.tensor_copy(qb[:], pss[(t, m)][:])
                            qr = rot_evac(pss[(t, m)], f"tr{js[t]}_{m}")
                            rope_math2(qb[:], qr, sqs[t])
                            qbs_j[t][m] = qb
                        emit_one_stage()

                # K projection for both blocks
                psk = {}
                for m in range(NKVH):
                    for t in range(2):
                        psk[(t, m)] = ps.tile([128, 512], F32, tag="ps",
                                              name=f"pk{js[t]}_{m}")
                for k in range(KT):
                    st, sp = (k == 0), (k == KT - 1)
                    for m in range(NKVH):
                        w = wk[:, k * 256 + m * 128: k * 256 + (m + 1) * 128]
                        for t in range(2):
                            nc.tensor.matmul(
                                psk[(t, m)][:], w,
                                xpair[t][:, k * 512:(k + 1) * 512],
                                start=st, stop=sp)
                for m in range(NKVH):
                    for t in range(2):
                        kdst = kt[:, m * S + js[t] * 512: m * S + (js[t] + 1) * 512]
                        nc.scalar.copy(kdst, psk[(t, m)][:])
                        qr = rot_evac(psk[(t, m)], f"trk{js[t]}_{m}")
                        rope_math2(kdst, qr, sqs[t])
                    emit_one_stage()

                # ---- V for both blocks (xt-stationary, per block) ----
                for t in range(2):
                    ps_v = [ps.tile([128, NKVH * HD], F32, tag="ps",
                                    name=f"pv{js[t]}_{u}")
                            for u in range(4)]
                    for k in range(KT):
                        st, sp = (k == 0), (k == KT - 1)
                        for u in range(4):
                            nc.tensor.matmul(
                                ps_v[u][:],
                                xpair[t][:, k * 512 + u * 128: k * 512 + (u + 1) * 128],
                                wv[:, k * 256:(k + 1) * 256],
                                start=st, stop=sp)
                    for u in range(4):
                        i = 4 * js[t] + u
                        nc.scalar.copy(vt[:, i * 256:(i + 1) * 256], ps_v[u][:])
                        emit_one_stage()

                if p == 1:
                    # reload weights for the next iteration; overlaps with
                    # this iteration's attention + o-proj
                    for c in range(4):
                        dma_w_chunk(wq, wq_d, NQH * HD, 5 * c, 5 * c + 5)
                        dma_w_chunk(wk, wk_d, NKVH * HD, 5 * c, 5 * c + 5)
                        dma_w_chunk(wv, wv_d, NKVH * HD, 5 * c, 5 * c + 5)

                # prefetch next phase's activations
                if p == 0:
                    xts[2] = xt_p.tile([128, KT * 512], BF16, tag="xt", name="xt2")
                    xts[3] = xt_p.tile([128, KT * 512], BF16, tag="xt", name="xt3")
                    for c in range(4):
                        dma_xt_chunk(xts[2], 2, 5 * c, 5 * c + 5)
                    for c in range(4):
                        dma_xt_chunk(xts[3], 3, 5 * c, 5 * c + 5)

                # flush the previous phase's ssq (frees its PSUM slots) and
                # interleave its o-proj chunks into this phase's attention
                if pend_oproj:
                    drain_tails()
                    pend_ssq_flush()

                def emit_oproj():
                    if pend_oproj:
                        pend_oproj.pop(0)()

                # ---- attention ----
                tws_j = [[], []]
                ssq_list = []
                for t in range(2):
                    ssq_list.append(attention_block(js[t], qbs_j[t], tws_j[t],
                                                    emit_oproj))
                while pend_oproj:
                    pend_oproj.pop(0)()

                # ---- o-proj chunk closures for this phase (wo shared) ----
                def mk_ssq_flush(ssqs, sqs_):
                    def f():
                        for t in range(2):
                            srow = mis_p.tile([1, 512], F32, tag="srow",
                                              name=f"sr{sqs_[t].start}", bufs=2)
                            nc.scalar.copy(srow[:], ssqs[t][:])
                            nc.sync.dma_start(ssq_d.ap()[:, sqs_[t]], srow[:])
                    return f

                pend_ssq_flush = mk_ssq_flush(ssq_list, sqs)

                def mk_chunk(m, tws_, sqs_, dve_only=True):
                    def f():
                        y_pss = [ps.tile([128, 512], F32, tag="ps",
                                         name=f"py{sqs_[t].start}_{m}")
                                 for t in range(2)]
                        for h in range(NQH):
                            w = wo[:, h * HID + m * 128: h * HID + (m + 1) * 128]
                            for t in range(2):
                                nc.tensor.matmul(
                                    y_pss[t][:], w, tws_[t][h][:],
                                    start=(h == 0), stop=(h == NQH - 1))
                        for t in range(2):
                            ysb = y_p.tile([128, 512], BF16, tag="ysb",
                                           name=f"y{sqs_[t].start}_{m}")
                            if (m + t) % 2 == 0 and not dve_only:
                                nc.scalar.copy(ysb[:], y_pss[t][:])
                            else:
                                nc.vector.tensor_copy(ysb[:], y_pss[t][:])
                            nc.sync.dma_start(
                                y_d.ap()[m * 128:(m + 1) * 128, sqs_[t]], ysb[:])
                    return f

                pend_oproj = [mk_chunk(m, tws_j, sqs, dve_only=(p == 0))
                              for m in range(KT)]
                if p == 1:
                    # last phase: nothing to hide under; run now
                    drain_tails()
                    pend_ssq_flush()
                    while pend_oproj:
                        pend_oproj.pop(0)()
                    nc.sync.dma_start(
                        wo[:].rearrange("p (h o) -> p h o", h=NQH),
                        wo_d.ap().rearrange("(h p) o -> p h o", p=128))

        if repeats > 1:
            with tc.For_i(0, repeats) as _i:
                body(_i)
        else:
            body()

    nc.compile()
    return nc


def _unpack_ternary(packed: np.ndarray) -> np.ndarray:
    M, Kp = packed.shape
    nb = Kp // 32
    b = packed.reshape(M, nb, 32)
    f = np.stack([(b >> 6) & 3, (b >> 4) & 3, (b >> 2) & 3, b & 3], axis=2)
    return f.reshape(M, nb * 128).astype(np.float32) - 1.0


def _rope_tables():
    inv = 1.0 / (THETA ** (np.arange(0, HD, 2, dtype=np.float64) / HD))  # (64,)
    t = np.arange(S, dtype=np.float64)
    fr = t[None, :] * inv[:, None]          # (64, S)
    cos = np.concatenate([np.cos(fr), np.cos(fr)], axis=0)      # (128, S)
    sin = np.concatenate([-np.sin(fr), np.sin(fr)], axis=0)     # signed
    return cos.astype(np.float16), sin.astype(np.float16)


def _tri_mask():
    q = np.arange(128)[None, :]
    p = np.arange(HD)[:, None]
    return (q >= p).astype(ml_dtypes.bfloat16)


def make_in_maps(hidden_states, q_w, k_w, v_w, o_w, attn_norm_w):
    wq_f = _unpack_ternary(np.asarray(q_w))     # (2560, 2560)
    wk_f = _unpack_ternary(np.asarray(k_w))     # (640, 2560)
    wv_f = _unpack_ternary(np.asarray(v_w))     # (640, 2560)
    wo_f = _unpack_ternary(np.asarray(o_w))     # (2560, 2560) [out, in]
    cos, sin = _rope_tables()
    tri = _tri_mask()
    onc = np.ones((HD, 1), np.float32)
    onr = np.ones((1, HD), np.float32)
    wnorm = np.asarray(attn_norm_w, np.float32)
    hs = np.asarray(hidden_states)

    bf = ml_dtypes.bfloat16
    in_maps = []
    for c in range(N_CORES):
        b, g = c // 4, c % 4
        qheads = [4 * g, 4 * g + 1, 4 * g + 2, 4 * g + 3, 16 + g]
        kvheads = [g, 4]
        qrows = np.concatenate([wq_f[h * HD:(h + 1) * HD] for h in qheads], 0)
        krows = np.concatenate([wk_f[h * HD:(h + 1) * HD] for h in kvheads], 0)
        vrows = np.concatenate([wv_f[h * HD:(h + 1) * HD] for h in kvheads], 0)
        ocols = np.concatenate([wo_f[:, h * HD:(h + 1) * HD] for h in qheads], 1)
        wn = np.stack([wnorm[h * HD:(h + 1) * HD] for h in qheads], 1)  # (128, 5)
        in_maps.append({
            "xt": np.ascontiguousarray(hs[b].T).astype(bf),
            "wq": np.ascontiguousarray(qrows.T).astype(bf),
            "wk": np.ascontiguousarray(krows.T).astype(bf),
            "wv": np.ascontiguousarray(vrows.T).astype(bf),
            "wo": np.ascontiguousarray(ocols.T).astype(bf),
            "cos": cos, "sin": sin,
            "wn": np.ascontiguousarray(wn),
            "tri": tri, "onc": onc, "onr": onr,
        })
    return in_maps


def postprocess(results, v_scale, o_scale):
    out = np.empty((B, S, HID), np.float32)
    for b in range(B):
        y = np.zeros((HID, S), np.float64)
        ss = np.zeros((S,), np.float64)
        for g in range(4):
            r = results[b * 4 + g]
            y += r["y"].astype(np.float64)
            ss += r["ssq"][0].astype(np.float64)
        var = ss * (float(v_scale) ** 2) / HID
        rms = 1.0 / np.sqrt(var + RMS_EPS)
        out[b] = (y.T * (rms[:, None] * float(v_scale) * float(o_scale))).astype(np.float32)
    return out


def _get_nc(alpha: float, repeats: int = 1):
    key = (round(alpha, 12), repeats)
    if key not in _cache:
        _cache[key] = _build(alpha, repeats)
    return _cache[key]


def kernel(hidden_states, attention_mask, q_w, k_w, v_w, o_w,
           q_scale, k_scale, v_scale, o_scale, attn_norm_w):
    alpha = float(q_scale) * float(k_scale) / math.sqrt(HD)
    nc = _get_nc(alpha, 1)
    in_maps = make_in_maps(hidden_states, q_w, k_w, v_w, o_w, attn_norm_w)
    res = bass_utils.run_bass_kernel_spmd(nc, in_maps, core_ids=list(range(N_CORES)))
    return postprocess(res.results, v_scale, o_scale)


# revision 5
# speedup vs baseline: 1.0626x; 1.0055x over previous
"""BitNet attention (B=2, S=2048, HID=2560, NH=20, NKV=5, HD=128, GQA=4) on 8 TRN2 cores.

v2: same sharding as baseline (2-way batch x 4-way head-group tensor parallel;
core (b,g) owns q-heads [4g..4g+3, 16+g], kv slots [g, 4]), restructured for
PE stationary-weight reuse and causal narrowing:
  - j-blocks processed in phases of 2: QKV projections and o-proj loop over
    both blocks per weight tile, so consecutive matmuls share the stationary
    operand (the second LDWEIGHTS of a repeated weight is much cheaper on HW).
  - attention processes kv0's q-heads in pairs: S/AV/d matmuls for both heads
    are emitted back-to-back sharing the K/V tile stationary.
  - diagonal 128-col tiles are narrowed: S/exp/AV/d only touch the causally
    live columns; the triangle mask shrinks to a single [128,128] constant.
  - softmax denominator rows for a head pair share one PSUM bank (partitions
    0 and 32).
Host: unpack ternary weights, build RoPE tables, sum partial y / sumsq over the
4 cores of each batch, apply v/o scales and the RMSNorm per-seq scale.
"""

import math
import numpy as np
import ml_dtypes
from contextlib import ExitStack

import concourse.bacc as bacc
import concourse.tile as tile
import concourse.mybir as mybir
from concourse import bass_utils

B, S, HID = 2, 2048, 2560
NH, NKV, HD = 20, 5, 128
THETA = 500000.0
RMS_EPS = 1e-6

N_CORES = 8
KT = HID // 128          # 20 k-tiles over hidden dim
J = S // 512             # 4 seq blocks of 512
SKT = S // 128           # 16 sk tiles
NQH = 5                  # q heads per core
NKVH = 2                 # kv heads per core

F32 = mybir.dt.float32
F32R = mybir.dt.float32r
BF16 = mybir.dt.bfloat16
F16 = mybir.dt.float16
FP8 = mybir.dt.float8e4

_cache = {}

DEPTH = 3  # AV flush queue depth (in tile steps)


def _build(alpha: float, repeats: int):
    nc = bacc.Bacc("TRN2", target_bir_lowering=False, debug=False, num_devices=N_CORES)

    xt_d = nc.dram_tensor("xt", [HID, S], BF16, kind="ExternalInput")
    wq_d = nc.dram_tensor("wq", [HID, NQH * HD], mybir.dt.uint8, kind="ExternalInput")
    wk_d = nc.dram_tensor("wk", [HID, NKVH * HD], mybir.dt.uint8, kind="ExternalInput")
    wv_d = nc.dram_tensor("wv", [HID, NKVH * HD], BF16, kind="ExternalInput")
    wo_d = nc.dram_tensor("wo", [NQH * HD, HID], mybir.dt.uint8, kind="ExternalInput")
    cos_d = nc.dram_tensor("cos", [HD, S], F16, kind="ExternalInput")
    sin_d = nc.dram_tensor("sin", [HD, S], F16, kind="ExternalInput")
    wn_d = nc.dram_tensor("wn", [HD, NQH], F32, kind="ExternalInput")
    tri_d = nc.dram_tensor("tri", [HD, 128], BF16, kind="ExternalInput")
    onc_d = nc.dram_tensor("onc", [HD, 1], F32R, kind="ExternalInput")
    onr_d = nc.dram_tensor("onr", [1, HD], F32R, kind="ExternalInput")
    y_d = nc.dram_tensor("y", [HID, S], BF16, kind="ExternalOutput")
    ssq_d = nc.dram_tensor("ssq", [1, S], F32, kind="ExternalOutput")

    with tile.TileContext(nc) as tc, ExitStack() as octx:
        ps = octx.enter_context(tc.tile_pool(name="ps", bufs=8, space="PSUM"))
        kt_p = octx.enter_context(tc.tile_pool(name="ktp", bufs=1))
        v_p = octx.enter_context(tc.tile_pool(name="vp", bufs=1))
        qb_p = octx.enter_context(tc.tile_pool(name="qbp", bufs=10))
        const_p = octx.enter_context(tc.tile_pool(name="constp", bufs=1))
        w_p = octx.enter_context(tc.tile_pool(name="wp", bufs=1))
        xt_p = octx.enter_context(tc.tile_pool(name="xtp", bufs=3))
        rp_p = octx.enter_context(tc.tile_pool(name="rpp", bufs=6))
        pr_p = octx.enter_context(tc.tile_pool(name="prp", bufs=8))
        tw_p = octx.enter_context(tc.tile_pool(name="twp", bufs=21))
        mis_p = octx.enter_context(tc.tile_pool(name="misp", bufs=4))
        y_p = octx.enter_context(tc.tile_pool(name="yp", bufs=4))

        def dma_w_chunk(dst, src_d, W, k0, k1):
            nc.sync.dma_start(
                dst[:, k0 * W:k1 * W].rearrange("p (k o) -> p k o", k=k1 - k0),
                src_d.ap()[k0 * 128:k1 * 128].rearrange("(k p) o -> p k o", p=128))

        # ---- prologue: weights + constants loaded once before the loop;
        # inside the loop the weight tiles are re-filled late (after their
        # last reader) so iteration N+1's loads overlap iteration N's tail.
        wq = w_p.tile([128, KT * NQH * HD], mybir.dt.uint8, tag="wq", name="wq")
        wk = w_p.tile([128, KT * NKVH * HD], mybir.dt.uint8, tag="wk", name="wk")
        wv = w_p.tile([128, KT * NKVH * HD], BF16, tag="wv", name="wv")
        wo = w_p.tile([128, NQH * HID], mybir.dt.uint8, tag="wo", name="wo")
        for c in range(4):
            dma_w_chunk(wq, wq_d, NQH * HD, 5 * c, 5 * c + 5)
            dma_w_chunk(wk, wk_d, NKVH * HD, 5 * c, 5 * c + 5)
            dma_w_chunk(wv, wv_d, NKVH * HD, 5 * c, 5 * c + 5)
        nc.sync.dma_start(
            wo[:].rearrange("p (h o) -> p h o", h=NQH),
            wo_d.ap().rearrange("(h p) o -> p h o", p=128))
        cos_t = const_p.tile([HD, S], F16, tag="cos", name="cos")
        nc.sync.dma_start(cos_t[:], cos_d.ap())
        sin_t = const_p.tile([HD, S], F16, tag="sin", name="sin")
        nc.sync.dma_start(sin_t[:], sin_d.ap())
        onc = const_p.tile([HD, 1], F32R, tag="onc", name="onc")
        nc.sync.dma_start(onc[:], onc_d.ap())
        onr = const_p.tile([1, HD], F32R, tag="onr", name="onr")
        nc.sync.dma_start(onr[:], onr_d.ap())
        wn = const_p.tile([HD, NQH], F32, tag="wn", name="wn")
        nc.sync.dma_start(wn[:], wn_d.ap())
        tri = const_p.tile([HD, 128], BF16, tag="tri", name="tri")
        nc.sync.dma_start(tri[:], tri_d.ap())
        onc_bf = const_p.tile([HD, 1], BF16, tag="oncb", name="oncb")
        nc.any.memset(onc_bf[:], 1.0)

        def body(_it=None):
            # --- per-iteration SBUF ---
            kt = kt_p.tile([128, NKVH * S], BF16, tag="kt", name="kt")
            vt = v_p.tile([128, SKT * NKVH * HD], BF16, tag="vt", name="vt")

            def dma_xt_chunk(dst, j, k0, k1):
                nc.sync.dma_start(
                    dst[:, k0 * 512:k1 * 512].rearrange("p (k s) -> p k s", k=k1 - k0),
                    xt_d.ap()[k0 * 128:k1 * 128, j * 512:(j + 1) * 512]
                    .rearrange("(k p) s -> p k s", p=128))

            xts = [None] * J
            xts[0] = xt_p.tile([128, KT * 512], BF16, tag="xt", name="xt0")
            xts[1] = xt_p.tile([128, KT * 512], BF16, tag="xt", name="xt1")
            for c in range(4):
                dma_xt_chunk(xts[0], 0, 5 * c, 5 * c + 5)
            for c in range(4):
                dma_xt_chunk(xts[1], 1, 5 * c, 5 * c + 5)

            # ---- tail pipeline: per-head FIFO through stages A->BC->B->C ----
            # A: drow copy (ACT) frees the packed d rows
            # BC: broadcast matmul (PE, fp32r self-loading)
            # B: reciprocal + normalize (DVE) - frees av + dbc psum slots
            # C: square (ACT) + ssq matmul (PE) + tw scale (DVE)
            pendA, pendBC, pendB, pendC = [], [], [], []

            def emit_tail_a():
                if not pendA:
                    return
                h, j, dsl, av_ps, ssq_ps, tws = pendA.pop(0)
                drow = mis_p.tile([1, 512], F32R, tag="drow", name=f"dr{j}_{h}",
                                  bufs=3)
                nc.scalar.copy(drow[:], dsl)
                pendBC.append((h, j, drow, av_ps, ssq_ps, tws))

            def emit_tail_bc():
                if not pendBC:
                    return
                h, j, drow, av_ps, ssq_ps, tws = pendBC.pop(0)
                dbc = ps.tile([128, 512], F32, tag="ps", name=f"db{j}_{h}")
                nc.tensor.matmul(dbc[:], onr[:], drow[:], start=True, stop=True)
                pendB.append((h, j, dbc, av_ps, ssq_ps, tws))

            def emit_tail_b():
                if not pendB:
                    return
                h, j, dbc, av_ps, ssq_ps, tws = pendB.pop(0)
                rec = mis_p.tile([128, 512], F32, tag="rec", name=f"rc{j}_{h}", bufs=2)
                nc.vector.reciprocal(rec[:], dbc[:])
                tn = mis_p.tile([128, 512], F32, tag="tn", name=f"tn{j}_{h}", bufs=3)
                nc.vector.tensor_mul(tn[:], av_ps[:], rec[:])
                pendC.append((h, j, tn, ssq_ps, tws))

            def emit_tail_c():
                if not pendC:
                    return
                h, j, tn, ssq_ps, tws = pendC.pop(0)
                sqt = mis_p.tile([128, 512], F32R, tag="sqt", name=f"sq{j}_{h}", bufs=2)
                nc.scalar.square(sqt[:], tn[:])
                nc.tensor.matmul(ssq_ps[:], onc[:], sqt[:],
                                 start=(h == 0), stop=(h == NQH - 1))
                tw = tw_p.tile([128, 512], BF16, tag="tw", name=f"tw{j}_{h}")
                nc.vector.tensor_scalar_mul(tw[:], tn[:], wn[:, h:h + 1])
                tws.append(tw)

            def emit_one_stage():
                # advance the oldest item one stage; at most one op per call
                if pendA:
                    emit_tail_a()
                elif pendBC:
                    emit_tail_bc()
                elif pendB:
                    emit_tail_b()
                elif pendC:
                    emit_tail_c()

            def drain_tails():
                while pendA or pendBC or pendB or pendC:
                    emit_one_stage()

            def rot_evac(psrc, nm):
                # rotate-half via two ACT cross-partition copies (PSUM->SBUF)
                qr = rp_p.tile([128, 512], F16, tag="trot", name=nm)
                nc.scalar.copy(qr[0:64, :], psrc[64:128, :])
                nc.scalar.copy(qr[64:128, :], psrc[0:64, :])
                return qr

            def rope_math2(dst, qr, sq):
                nc.vector.tensor_mul(dst, dst, cos_t[:, sq])
                nc.vector.tensor_mul(qr[:], qr[:], sin_t[:, sq])
                nc.vector.tensor_add(dst, dst, qr[:])

            # ================= attention for one block =================
            def attention_block(j, qbs, tws, emit_extra=None):
                ni = 4 * j + 4
                sq = slice(j * 512, (j + 1) * 512)
                ssq_ps = ps.tile([1, 512], F32, tag="ps", name=f"pss{j}")

                for pair in ((0, 1), (2, 3), (4,)):
                    kvl = 0 if pair[0] < 4 else 1
                    avs = [ps.tile([128, 512], F32, tag="ps", name=f"pav{j}_{h}")
                           for h in pair]
                    dt = ps.tile([128, 512], F32, tag="ps", name=f"pd{j}_{pair[0]}")
                    drows = [dt[32 * z:32 * z + 1, :] for z in range(len(pair))]
                    queue = []

                    def flush_one():
                        pi, plist = queue.pop(0)
                        st, sp = (pi == 0), (pi == ni - 1)
                        for z, (pr, cs) in enumerate(plist):
                            nc.tensor.matmul(
                                avs[z][:, cs],
                                vt[:, pi * 256 + kvl * 128: pi * 256 + kvl * 128 + 128],
                                pr[:, cs], start=st, stop=sp)
                        for z, (pr, cs) in enumerate(plist):
                            nc.tensor.matmul(
                                drows[z][:, cs], onc_bf[:], pr[:, cs],
                                start=st, stop=sp)

                    for i in range(ni):
                        o = i - 4 * j
                        cs = slice(128 * o, 512) if o > 0 else slice(0, 512)
                        plist = []
                        for z, h in enumerate(pair):
                            s_ps = ps.tile([128, 512], F32, tag="ps",
                                           name=f"pS{j}_{h}_{i}")
                            nc.tensor.matmul(
                                s_ps[:, cs],
                                kt[:, kvl * S + i * 128: kvl * S + (i + 1) * 128],
                                qbs[h][:, cs], start=True, stop=True)
                            probs = pr_p.tile([128, 512], BF16, tag="probs",
                                              name=f"pr{j}_{h}_{i}")
                            nc.scalar.activation(
                                probs[:, cs], s_ps[:, cs],
                                mybir.ActivationFunctionType.Exp, scale=alpha)
                            if o >= 0:
                                nc.vector.tensor_mul(
                                    probs[:, 128 * o:128 * o + 128],
                                    probs[:, 128 * o:128 * o + 128], tri[:])
                            plist.append((probs, cs))
                        queue.append((i, plist))
                        if len(queue) > DEPTH:
                            flush_one()
                        # tails of previous heads, up to 2 stage-ops per step
                        emit_one_stage()
                        emit_one_stage()
                        if emit_extra is not None:
                            emit_extra()
                    while queue:
                        flush_one()
                    for z, h in enumerate(pair):
                        pendA.append((h, j, drows[z], avs[z], ssq_ps, tws))

                return ssq_ps

            # ================= phases =================
            pend_oproj = []
            pend_ssq_flush = None
            for p in range(2):
                js = (2 * p, 2 * p + 1)
                xpair = [xts[js[0]], xts[js[1]]]
                sqs = [slice(jj * 512, (jj + 1) * 512) for jj in js]

                # ---- Q/K projections, grouped; weight stationary shared
                # across the two blocks ----
                qbs_j = [[None] * NQH for _ in range(2)]
                for group in ((0, 1, 2), (3, 4)):
                    pss = {}
                    for m in group:
                        for t in range(2):
                            pss[(t, m)] = ps.tile([128, 512], F32, tag="ps",
                                                  name=f"pq{js[t]}_{m}")
                    for k in range(KT):
                        st, sp = (k == 0), (k == KT - 1)
                        for m in group:
                            w = wq[:, k * 640 + m * 128: k * 640 + (m + 1) * 128].bitcast(FP8)
                            for t in range(2):
                                nc.tensor.matmul(
                                    pss[(t, m)][:], w,
                                    xpair[t][:, k * 512:(k + 1) * 512],
                                    start=st, stop=sp)
                        if p == 0 and group[0] == 0 and k < 8:
                            emit_one_stage()
                    for m in group:
                        for t in range(2):
                            qb = qb_p.tile([128, 512], BF16, tag="qb",
                                           name=f"qb{js[t]}_{m}")
                            nc.vector.tensor_copy(qb[:], pss[(t, m)][:])
                            qr = rot_evac(pss[(t, m)], f"tr{js[t]}_{m}")
                            rope_math2(qb[:], qr, sqs[t])
                            qbs_j[t][m] = qb
                        emit_one_stage()

                # K projection for both blocks
                psk = {}
                for m in range(NKVH):
                    for t in range(2):
                        psk[(t, m)] = ps.tile([128, 512], F32, tag="ps",
                                              name=f"pk{js[t]}_{m}")
                for k in range(KT):
                    st, sp = (k == 0), (k == KT - 1)
                    for m in range(NKVH):
                        w = wk[:, k * 256 + m * 128: k * 256 + (m + 1) * 128].bitcast(FP8)
                        for t in range(2):
                            nc.tensor.matmul(
                                psk[(t, m)][:], w,
                                xpair[t][:, k * 512:(k + 1) * 512],
                                start=st, stop=sp)
                for m in range(NKVH):
                    for t in range(2):
                        kdst = kt[:, m * S + js[t] * 512: m * S + (js[t] + 1) * 512]
                        nc.scalar.copy(kdst, psk[(t, m)][:])
                        qr = rot_evac(psk[(t, m)], f"trk{js[t]}_{m}")
                        rope_math2(kdst, qr, sqs[t])
                    emit_one_stage()

                # ---- V for both blocks (xt-stationary, per block) ----
                for t in range(2):
                    ps_v = [ps.tile([128, NKVH * HD], F32, tag="ps",
                                    name=f"pv{js[t]}_{u}")
                            for u in range(4)]
                    for k in range(KT):
                        st, sp = (k == 0), (k == KT - 1)
                        for u in range(4):
                            nc.tensor.matmul(
                                ps_v[u][:],
                                xpair[t][:, k * 512 + u * 128: k * 512 + (u + 1) * 128],
                                wv[:, k * 256:(k + 1) * 256],
                                start=st, stop=sp)
                    for u in range(4):
                        i = 4 * js[t] + u
                        nc.scalar.copy(vt[:, i * 256:(i + 1) * 256], ps_v[u][:])
                        emit_one_stage()

                if p == 1:
                    # reload weights for the next iteration; overlaps with
                    # this iteration's attention + o-proj
                    for c in range(4):
                        dma_w_chunk(wq, wq_d, NQH * HD, 5 * c, 5 * c + 5)
                        dma_w_chunk(wk, wk_d, NKVH * HD, 5 * c, 5 * c + 5)
                        dma_w_chunk(wv, wv_d, NKVH * HD, 5 * c, 5 * c + 5)

                # prefetch next phase's activations
                if p == 0:
                    xts[2] = xt_p.tile([128, KT * 512], BF16, tag="xt", name="xt2")
                    xts[3] = xt_p.tile([128, KT * 512], BF16, tag="xt", name="xt3")
                    for c in range(4):
                        dma_xt_chunk(xts[2], 2, 5 * c, 5 * c + 5)
                    for c in range(4):
                        dma_xt_chunk(xts[3], 3, 5 * c, 5 * c + 5)

                # flush the previous phase's ssq (frees its PSUM slots) and
                # interleave its o-proj chunks into this phase's attention
                if pend_oproj:
                    drain_tails()
                    pend_ssq_flush()

                def emit_oproj():
                    if pend_oproj:
                        pend_oproj.pop(0)()

                # ---- attention ----
                tws_j = [[], []]
                ssq_list = []
                for t in range(2):
                    ssq_list.append(attention_block(js[t], qbs_j[t], tws_j[t],
                                                    emit_oproj))
                while pend_oproj:
                    pend_oproj.pop(0)()

                # ---- o-proj chunk closures for this phase (wo shared) ----
                def mk_ssq_flush(ssqs, sqs_):
                    def f():
                        for t in range(2):
                            srow = mis_p.tile([1, 512], F32, tag="srow",
                                              name=f"sr{sqs_[t].start}", bufs=2)
                            nc.scalar.copy(srow[:], ssqs[t][:])
                            nc.sync.dma_start(ssq_d.ap()[:, sqs_[t]], srow[:])
                    return f

                pend_ssq_flush = mk_ssq_flush(ssq_list, sqs)

                def mk_chunk(m, tws_, sqs_, dve_only=True):
                    def f():
                        y_pss = [ps.tile([128, 512], F32, tag="ps",
                                         name=f"py{sqs_[t].start}_{m}")
                                 for t in range(2)]
                        for h in range(NQH):
                            w = wo[:, h * HID + m * 128: h * HID + (m + 1) * 128].bitcast(FP8)
                            for t in range(2):
                                nc.tensor.matmul(
                                    y_pss[t][:], w, tws_[t][h][:],
                                    start=(h == 0), stop=(h == NQH - 1))
                        for t in range(2):
                            ysb = y_p.tile([128, 512], BF16, tag="ysb",
                                           name=f"y{sqs_[t].start}_{m}")
                            if (m + t) % 2 == 0 and not dve_only:
                                nc.scalar.copy(ysb[:], y_pss[t][:])
                            else:
                                nc.vector.tensor_copy(ysb[:], y_pss[t][:])
                            nc.sync.dma_start(
                                y_d.ap()[m * 128:(m + 1) * 128, sqs_[t]], ysb[:])
                    return f

                pend_oproj = [mk_chunk(m, tws_j, sqs, dve_only=(p == 0))
                              for m in range(KT)]
                if p == 1:
                    # last phase: nothing to hide under; run now
                    drain_tails()
                    pend_ssq_flush()
                    while pend_oproj:
                        pend_oproj.pop(0)()
                    nc.sync.dma_start(
                        wo[:].rearrange("p (h o) -> p h o", h=NQH),
                        wo_d.ap().rearrange("(h p) o -> p h o", p=128))

        if repeats > 1:
            with tc.For_i(0, repeats) as _i:
                body(_i)
        else:
            body()

    nc.compile()
    return nc


def _unpack_ternary(packed: np.ndarray) -> np.ndarray:
    M, Kp = packed.shape
    nb = Kp // 32
    b = packed.reshape(M, nb, 32)
    f = np.stack([(b >> 6) & 3, (b >> 4) & 3, (b >> 2) & 3, b & 3], axis=2)
    return f.reshape(M, nb * 128).astype(np.float32) - 1.0


def _rope_tables():
    inv = 1.0 / (THETA ** (np.arange(0, HD, 2, dtype=np.float64) / HD))  # (64,)
    t = np.arange(S, dtype=np.float64)
    fr = t[None, :] * inv[:, None]          # (64, S)
    cos = np.concatenate([np.cos(fr), np.cos(fr)], axis=0)      # (128, S)
    sin = np.concatenate([-np.sin(fr), np.sin(fr)], axis=0)     # signed
    return cos.astype(np.float16), sin.astype(np.float16)


def _tri_mask():
    q = np.arange(128)[None, :]
    p = np.arange(HD)[:, None]
    return (q >= p).astype(ml_dtypes.bfloat16)


def make_in_maps(hidden_states, q_w, k_w, v_w, o_w, attn_norm_w):
    wq_f = _unpack_ternary(np.asarray(q_w))     # (2560, 2560)
    wk_f = _unpack_ternary(np.asarray(k_w))     # (640, 2560)
    wv_f = _unpack_ternary(np.asarray(v_w))     # (640, 2560)
    wo_f = _unpack_ternary(np.asarray(o_w))     # (2560, 2560) [out, in]
    cos, sin = _rope_tables()
    tri = _tri_mask()
    onc = np.ones((HD, 1), np.float32)
    onr = np.ones((1, HD), np.float32)
    wnorm = np.asarray(attn_norm_w, np.float32)
    hs = np.asarray(hidden_states)

    bf = ml_dtypes.bfloat16
    in_maps = []
    for c in range(N_CORES):
        b, g = c // 4, c % 4
        qheads = [4 * g, 4 * g + 1, 4 * g + 2, 4 * g + 3, 16 + g]
        kvheads = [g, 4]
        qrows = np.concatenate([wq_f[h * HD:(h + 1) * HD] for h in qheads], 0)
        krows = np.concatenate([wk_f[h * HD:(h + 1) * HD] for h in kvheads], 0)
        vrows = np.concatenate([wv_f[h * HD:(h + 1) * HD] for h in kvheads], 0)
        ocols = np.concatenate([wo_f[:, h * HD:(h + 1) * HD] for h in qheads], 1)
        wn = np.stack([wnorm[h * HD:(h + 1) * HD] for h in qheads], 1)  # (128, 5)
        in_maps.append({
            "xt": np.ascontiguousarray(hs[b].T).astype(bf),
            "wq": np.ascontiguousarray(qrows.T).astype(ml_dtypes.float8_e4m3fn).view(np.uint8),
            "wk": np.ascontiguousarray(krows.T).astype(ml_dtypes.float8_e4m3fn).view(np.uint8),
            "wv": np.ascontiguousarray(vrows.T).astype(bf),
            "wo": np.ascontiguousarray(ocols.T).astype(ml_dtypes.float8_e4m3fn).view(np.uint8),
            "cos": cos, "sin": sin,
            "wn": np.ascontiguousarray(wn),
            "tri": tri, "onc": onc, "onr": onr,
        })
    return in_maps


def postprocess(results, v_scale, o_scale):
    out = np.empty((B, S, HID), np.float32)
    for b in range(B):
        y = np.zeros((HID, S), np.float64)
        ss = np.zeros((S,), np.float64)
        for g in range(4):
            r = results[b * 4 + g]
            y += r["y"].astype(np.float64)
            ss += r["ssq"][0].astype(np.float64)
        var = ss * (float(v_scale) ** 2) / HID
        rms = 1.0 / np.sqrt(var + RMS_EPS)
        out[b] = (y.T * (rms[:, None] * float(v_scale) * float(o_scale))).astype(np.float32)
    return out


def _get_nc(alpha: float, repeats: int = 1):
    key = (round(alpha, 12), repeats)
    if key not in _cache:
        _cache[key] = _build(alpha, repeats)
    return _cache[key]


def kernel(hidden_states, attention_mask, q_w, k_w, v_w, o_w,
           q_scale, k_scale, v_scale, o_scale, attn_norm_w):
    alpha = float(q_scale) * float(k_scale) / math.sqrt(HD)
    nc = _get_nc(alpha, 1)
    in_maps = make_in_maps(hidden_states, q_w, k_w, v_w, o_w, attn_norm_w)
    res = bass_utils.run_bass_kernel_spmd(nc, in_maps, core_ids=list(range(N_CORES)))
    return postprocess(res.results, v_scale, o_scale)
